# revision 5
# baseline (speedup 1.0000x reference)
"""Multi-head attention (B=2, N=2048, E=1024, H=16) on 8 Trainium2 NeuronCores.

Sharding: data-parallel over batch (2) x tensor-parallel over head-groups (4
groups of 4 heads).  Core c handles batch c//4 and heads 4*(c%4)..4*(c%4)+3.

Host-side shard prep packs ALL per-core inputs (feature-major fp16
activations, transposed fp16 weight shards, fp16 biases) into a single flat
fp16 blob — one input parameter + one output parameter minimizes the
per-parameter dispatch cost of each execution.  The device kernel computes
  qT = Wq_s @ xT + bq_s        (feature-major, [256, 2048], fp16)
  kT = Wk_s @ xT + bk_s
  v  = x @ Wv_s.T + bv_s       (position-major, [2048, 256], bf16)
  eT[kpos, q] per head          (transposed energy, head pairs row-packed
                                 into the PE array, K=64 each)
  s = exp(eT)  (bf16)           (no max-subtraction: |logits| < ~60 << 88)
  o  = s.T @ [v | 32]           (32-column yields 32*rowsum in psum row 64)
  oT normalized by 1/(32*rowsum)   (= softmax / sqrt(E) module quirk)
  out_partial = oT.T @ Wp[:, cols].T   (position-major [2048, 1024], fp16)
Host sums the 4 head-group partials per batch and adds bp.

All matmuls run with 16-bit operands (fp16 where range allows, bf16 for the
attention weights whose unnormalized exp can reach ~e^50); PSUM accumulation
is fp32.  q-chunk projections and the output projection are emitted
*interleaved* with the attention inner loop so the PE fills the gaps while
ScalarE (exp) runs.
"""

import numpy as np

B, N, E, H = 2, 2048, 1024, 16
D = E // H           # 64
NCORES = 8
HG = 4               # head groups
DH = E // HG         # 256 features per head-group
P = 128
NCH = N // 512       # 4 n-chunks of 512
ECH = E // P         # 8 contraction chunks
DCH = DH // P        # 2 feature chunks per shard
KT = N // P          # 16 key tiles
SCALE_COL = float(E ** 0.5)   # 32.0; row 64 of po = 32*rowsum

# flat fp16 blob layout (element offsets).  Each core carries the FULL x
# for its batch (device-resident DRAM is plentiful and steady-state
# executions reuse it), stored n-chunk-major so each 512-position chunk is
# contiguous: [ni][ec][p][m] = xT[ec*128+p, ni*512+m].
SZ_X = E * N          # 2097152 full
SZ_XS = SZ_X // HG    # 524288 per 512-position chunk
SZ_W = E * DH         # 262144
SZ_WP = DH * E        # 262144
OFF_XQ = 0
OFF_XK = OFF_XQ + SZ_X
OFF_XV = OFF_XK + SZ_X
OFF_WQ = OFF_XV + SZ_X
OFF_WK = OFF_WQ + SZ_W
OFF_WV = OFF_WK + SZ_W
OFF_WP = OFF_WV + SZ_W
OFF_BQ = OFF_WP + SZ_WP
OFF_BK = OFF_BQ + DH
OFF_BV = OFF_BK + DH
SZ_BLOB = OFF_BV + DH

_CACHE = {}


def _build_program():
    import concourse.bacc as bacc
    import concourse.tile as tile
    from concourse import mybir

    F32 = mybir.dt.float32
    F16 = mybir.dt.float16
    BF16 = mybir.dt.bfloat16
    EXP = mybir.ActivationFunctionType.Exp

    nc = bacc.Bacc(None, target_bir_lowering=False, debug=False,
                   enable_partition_id=False, num_devices=NCORES)

    blob = nc.declare_dram_parameter("blob", [SZ_BLOB], F16, isOutput=False)
    out = nc.declare_dram_parameter("out", [N, E], F16, isOutput=True)

    # dram views into the blob
    wqt = blob[OFF_WQ : OFF_WQ + SZ_W].rearrange("(c p m) -> p c m", c=ECH, p=P)
    wkt = blob[OFF_WK : OFF_WK + SZ_W].rearrange("(c p m) -> p c m", c=ECH, p=P)
    wvt = blob[OFF_WV : OFF_WV + SZ_W].rearrange("(c p m) -> p c m", c=ECH, p=P)
    wpt = blob[OFF_WP : OFF_WP + SZ_WP].rearrange("(c p m) -> p c m", c=DCH, p=P)
    bqv = blob[OFF_BQ : OFF_BQ + DH].rearrange("(c p) -> p c", p=P)
    bkv = blob[OFF_BK : OFF_BK + DH].rearrange("(c p) -> p c", p=P)
    bvv = blob[OFF_BV : OFF_BV + DH].rearrange("(a m) -> a m", a=1)

    with tile.TileContext(nc) as tc:
        with (
            nc.allow_low_precision(reason="16-bit activations; tol 2e-2"),
            tc.tile_pool(name="singles", bufs=1) as singles,
            tc.tile_pool(name="xpool", bufs=3) as xpool,
            tc.tile_pool(name="spool", bufs=4) as spool,
            tc.tile_pool(name="npool", bufs=2) as npool,
            tc.tile_pool(name="opool", bufs=2) as opool,
            tc.tile_pool(name="pproj", bufs=1, space="PSUM") as pproj,
            tc.tile_pool(name="peps", bufs=2, space="PSUM") as peps,
            tc.tile_pool(name="ppo", bufs=2, space="PSUM") as ppo,
            tc.tile_pool(name="pbc", bufs=1, space="PSUM") as pbc,
        ):
            # x lives in the blob n-chunk-major: chunk ni holds positions
            # [ni*512, (ni+1)*512) feature-major and is contiguous.
            XOFF = {"xq": OFF_XQ, "xk": OFF_XK, "xv": OFF_XV}

            def x_chunk_ap(nm, ni):
                off = XOFF[nm] + ni * SZ_XS
                return blob[off : off + SZ_XS].rearrange(
                    "(c p m) -> p c m", c=ECH, p=P
                )

            # ---- persistent weights / biases ----
            wq_sb = singles.tile([P, ECH, DH], F16)
            wk_sb = singles.tile([P, ECH, DH], F16)
            wv_sb = singles.tile([P, ECH, DH], F16)
            wp_sb = singles.tile([P, DCH, E], F16)
            nc.sync.dma_start(out=wq_sb, in_=wqt)
            nc.sync.dma_start(out=wk_sb, in_=wkt)
            nc.sync.dma_start(out=wv_sb, in_=wvt)
            nc.sync.dma_start(out=wp_sb, in_=wpt)
            bq16 = singles.tile([P, DCH], F16)
            bk16 = singles.tile([P, DCH], F16)
            nc.sync.dma_start(out=bq16, in_=bqv)
            nc.sync.dma_start(out=bk16, in_=bkv)
            bq_sb = singles.tile([P, DCH], F32)
            bk_sb = singles.tile([P, DCH], F32)
            nc.vector.tensor_copy(bq_sb, bq16)
            nc.vector.tensor_copy(bk_sb, bk16)
            bv_sb = singles.tile([1, DH], F16)
            nc.sync.dma_start(out=bv_sb, in_=bvv)
            ones1 = singles.tile([1, P], F16)
            nc.vector.memset(ones1, 1.0)
            ones1_b = singles.tile([1, P], BF16)
            nc.vector.memset(ones1_b, 1.0)

            qT_sb = singles.tile([P, DCH, N], F16)
            kT_sb = singles.tile([P, DCH, N], F16)
            oT_sb = singles.tile([P, DCH, N], F16)
            v_sb = singles.tile([P, KT, HG, D + 1], BF16)
            nc.vector.memset(v_sb[:, :, :, D : D + 1], SCALE_COL)

            # ---- emit helpers ----
            def emit_kv_chunk(ni):
                ns = slice(ni * 512, (ni + 1) * 512)
                xk_c = xpool.tile([P, ECH, 512], F16, tag="x", name=f"xk{ni}")
                nc.sync.dma_start(out=xk_c, in_=x_chunk_ap("xk", ni))
                for dc in range(DCH):
                    ps = pproj.tile([P, 512], F32, tag="proj", name=f"kps{ni}{dc}")
                    for ec in range(ECH):
                        nc.tensor.matmul(
                            ps,
                            wk_sb[:, ec, dc * P : (dc + 1) * P],
                            xk_c[:, ec, :],
                            start=(ec == 0),
                            stop=(ec == ECH - 1),
                        )
                    nc.vector.tensor_scalar_add(
                        kT_sb[:, dc, ns], ps, bk_sb[:, dc : dc + 1]
                    )
                xv_c = xpool.tile([P, ECH, 512], F16, tag="x", name=f"xv{ni}")
                nc.sync.dma_start(out=xv_c, in_=x_chunk_ap("xv", ni))
                for k4 in range(4):
                    kt = ni * 4 + k4
                    vps = pproj.tile([P, DH], F32, tag="proj", name=f"vps{kt}")
                    nc.tensor.matmul(vps, ones1, bv_sb, start=True, stop=False)
                    for ec in range(ECH):
                        nc.tensor.matmul(
                            vps,
                            xv_c[:, ec, k4 * P : (k4 + 1) * P],
                            wv_sb[:, ec, :],
                            start=False,
                            stop=(ec == ECH - 1),
                        )
                    nc.vector.tensor_copy(
                        v_sb[:, kt, :, 0:D],
                        vps.rearrange("p (h d) -> p h d", h=HG),
                    )

            def q_proj_units(ni):
                """Deferred q-projection for chunk ni: DMA + one unit per dc."""
                ns = slice(ni * 512, (ni + 1) * 512)
                state = {}

                def dma_unit():
                    xq_c = xpool.tile([P, ECH, 512], F16, tag="x", name=f"xq{ni}")
                    nc.sync.dma_start(out=xq_c, in_=x_chunk_ap("xq", ni))
                    state["xq"] = xq_c

                def unit(dc):
                    xq_c = state["xq"]
                    ps = pproj.tile([P, 512], F32, tag="proj", name=f"qps{ni}{dc}")
                    for ec in range(ECH):
                        nc.tensor.matmul(
                            ps,
                            wq_sb[:, ec, dc * P : (dc + 1) * P],
                            xq_c[:, ec, :],
                            start=(ec == 0),
                            stop=(ec == ECH - 1),
                        )
                    nc.vector.tensor_scalar_add(
                        qT_sb[:, dc, ns], ps, bq_sb[:, dc : dc + 1]
                    )

                return [dma_unit] + [lambda dc=dc: unit(dc) for dc in range(DCH)]

            def outproj_units(qc):
                """Deferred output projection for q-chunk qc: 4 n-tile units."""

                def unit(nt):
                    n0 = qc * 512 + nt * P
                    osb = opool.tile([P, E], F16, tag="osb", name=f"osb{qc}{nt}")
                    for ecx in range(2):
                        ops = pproj.tile(
                            [P, 512], F32, tag="proj", name=f"ops{qc}{nt}{ecx}"
                        )
                        for dc in range(DCH):
                            nc.tensor.matmul(
                                ops,
                                oT_sb[:, dc, n0 : n0 + P],
                                wp_sb[:, dc, ecx * 512 : (ecx + 1) * 512],
                                start=(dc == 0),
                                stop=(dc == DCH - 1),
                            )
                        nc.vector.tensor_copy(
                            osb[:, ecx * 512 : (ecx + 1) * 512], ops
                        )
                    nc.sync.dma_start(out=out[n0 : n0 + P, :], in_=osb)

                return [lambda nt=nt: unit(nt) for nt in range(4)]

            def attn_groups(qc, pr, po, ktgs, slots=None, si0=0):
                qs = slice(qc * 512, (qc + 1) * 512)
                si = si0
                for ktg in ktgs:
                    eps = [
                        peps.tile([P, 1024], F32, tag="eps", name=f"eps{hp}")
                        for hp in range(2)
                    ]
                    for j in range(2):
                        kt = ktg * 2 + j
                        ks = slice(kt * P, (kt + 1) * P)
                        for hp in range(2):
                            rows = slice(hp * D, (hp + 1) * D)
                            nc.tensor.matmul(
                                eps[hp][:, j * 512 : (j + 1) * 512],
                                kT_sb[rows, pr, ks],
                                qT_sb[rows, pr, qs],
                                start=True,
                                stop=True,
                            )
                    sT = [
                        spool.tile([P, 1024], BF16, tag="sT", name=f"sT{hp}")
                        for hp in range(2)
                    ]
                    for hp in range(2):
                        nc.scalar.activation(sT[hp], eps[hp], EXP)
                    for j in range(2):
                        kt = ktg * 2 + j
                        for hp in range(2):
                            nc.tensor.matmul(
                                po[hp],
                                v_sb[:, kt, 2 * pr + hp, :],
                                sT[hp][:, j * 512 : (j + 1) * 512],
                                start=(kt == 0),
                                stop=(kt == KT - 1),
                            )
                    if slots is not None:
                        for u in slots[si]:
                            u()
                        si += 1

            def normalize(qc, pr, po):
                qs = slice(qc * 512, (qc + 1) * 512)
                for hp in range(2):
                    rinv = npool.tile([1, 512], BF16, tag="rinv")
                    nc.vector.reciprocal(rinv, po[hp][D : D + 1, :])
                    o_tmp = npool.tile([D, 512], F32, tag="otmp")
                    nc.vector.tensor_copy(o_tmp, po[hp][0:D, :])
                    bc = pbc.tile([D, 512], F32, tag="bc")
                    nc.tensor.matmul(
                        bc, ones1_b[:, 0:D], rinv, start=True, stop=True
                    )
                    nc.vector.tensor_mul(
                        oT_sb[hp * D : (hp + 1) * D, pr, qs], o_tmp, bc
                    )

            def new_po():
                return [
                    ppo.tile([D + 1, 512], F32, tag="po", name=f"po{hp}")
                    for hp in range(2)
                ]

            # ---- emission: interleave qc=0 attention into the k/v loads so
            # ScalarE starts exp'ing as soon as the first k/v tiles land ----
            emit_kv_chunk(0)
            for u in q_proj_units(0):
                u()
            emit_kv_chunk(1)
            po0 = new_po()
            attn_groups(0, 0, po0, range(0, 4))        # ktiles 0-7 (kv 0,1)
            emit_kv_chunk(2)
            attn_groups(0, 0, po0, range(4, 6))        # ktiles 8-11 (kv 2)
            emit_kv_chunk(3)
            attn_groups(0, 0, po0, range(6, 8))        # ktiles 12-15 (kv 3)
            normalize(0, 0, po0)
            # q-chunk-1 projection interleaved into qc0/pr1 attention
            d0 = q_proj_units(1)
            slots0 = [[] for _ in range(KT // 2)]
            for i, u in enumerate(d0):
                slots0[(i * (KT // 2)) // len(d0)].append(u)
            po1 = new_po()
            attn_groups(0, 1, po1, range(0, 8), slots0, 0)
            normalize(0, 1, po1)

            # ---- remaining q-chunks with deferred work interleaved ----
            for qc in range(1, NCH):
                deferred = q_proj_units(qc + 1) if qc + 1 < NCH else []
                deferred += outproj_units(qc - 1)
                nslots = DCH * (KT // 2)
                slots = [[] for _ in range(nslots)]
                for i, u in enumerate(deferred):
                    slots[(i * nslots) // max(len(deferred), 1)].append(u)
                for pr in range(DCH):
                    po = new_po()
                    attn_groups(qc, pr, po, range(KT // 2), slots,
                                pr * (KT // 2))
                    normalize(qc, pr, po)
            # tail: output projection of the last q-chunk
            for u in outproj_units(NCH - 1):
                u()

    nc.compile()
    return nc


def _shard_inputs(queries, keys, values, Wq, bq, Wk, bk, Wv, bv, Wp):
    """Host-side shard/layout prep: one flat fp16 blob per core holding
    the full feature-major activations for its batch (n-chunk-major),
    transposed weight shards, and biases."""
    f32 = np.float32
    f16 = np.float16
    # full x per batch: [ni][ec][p][m] = x[b].T[ec*128+p, ni*512+m]
    xCM = {}
    for name, x in (("xq", queries), ("xk", keys), ("xv", values)):
        xCM[name] = [
            np.ascontiguousarray(
                np.asarray(x[b], f32).T.astype(f16)
                .reshape(ECH, P, NCH, 512).transpose(2, 0, 1, 3)
            ).ravel()
            for b in range(B)
        ]
    Wq, Wk, Wv = (np.asarray(w, f32) for w in (Wq, Wk, Wv))
    Wp = np.asarray(Wp, f32)
    bq, bk, bv = (np.asarray(b_, f32) for b_ in (bq, bk, bv))
    maps = []
    for c in range(NCORES):
        b, hg = c // HG, c % HG
        rows = slice(hg * DH, (hg + 1) * DH)
        blob = np.concatenate([
            xCM["xq"][b], xCM["xk"][b], xCM["xv"][b],
            np.ascontiguousarray(Wq[rows].T.astype(f16)).ravel(),
            np.ascontiguousarray(Wk[rows].T.astype(f16)).ravel(),
            np.ascontiguousarray(Wv[rows].T.astype(f16)).ravel(),
            np.ascontiguousarray(Wp[:, rows].T.astype(f16)).ravel(),
            bq[rows].astype(f16), bk[rows].astype(f16), bv[rows].astype(f16),
        ])
        assert blob.shape[0] == SZ_BLOB
        maps.append({"blob": blob})
    return maps


def kernel(queries, keys, values, Wq, bq, Wk, bk, Wv, bv, Wp, bp):
    from concourse.bass_utils import run_bass_kernel_spmd

    if "nc" not in _CACHE:
        _CACHE["nc"] = _build_program()
    nc = _CACHE["nc"]

    in_maps = _shard_inputs(queries, keys, values, Wq, bq, Wk, bk, Wv, bv, Wp)

    res = run_bass_kernel_spmd(nc, in_maps, list(range(NCORES)))

    out = np.zeros((B, N, E), np.float32)
    for c in range(NCORES):
        out[c // HG] += res.results[c]["out"].astype(np.float32)
    out += np.asarray(bp, np.float32)
    return out



# revision 25
# speedup vs baseline: 2.1893x; 2.1893x over previous
"""Multi-head attention (B=2, N=2048, E=1024, H=16) on 8 Trainium2 NeuronCores.

Sharding: data-parallel over batch (2) x tensor-parallel over head-groups (4
groups of 4 heads).  Core c handles batch c//4 and heads 4*(c%4)..4*(c%4)+3.

Each core receives the FULL activations for its batch in one flat fp16 blob
(n-chunk-major feature-major x, transposed fp16 weight shards, biases) --
device DRAM is plentiful and steady-state executions reuse device-resident
inputs, so no collectives are needed.

Device kernel per core (feature-major fp16 activations):
  kT = Wk_s @ xT + bk_s           [256, 2048] fp16
  qT = Wq_s @ xT + bq_s           [256, 2048] fp16
  v  = x @ Wv_s.T + bv_s          [2048, 256] bf16, plus a 65th column = 32.0
  per 512-query round (qc, pr):  eT[k, q] per head pair -> exp (bf16, no max
    subtraction: |logits| < ~60 << 88) staged in SBUF for the NEXT round;
  AV (flipped): po[q, 65] = sum_k s[k, q-tile].T v[k, :]  -- the 65th column
    accumulates 32*rowsum, so softmax/sqrt(E) normalization is a per-partition
    reciprocal+scale on DVE;  o[q, d] -> oT[d, q] via DMA xbar transpose.
  out_partial = oT.T @ Wp[:, cols].T    [2048, 1024] fp16
Host sums the 4 head-group partials per batch and adds bp.

All matmuls use 16-bit operands; PSUM is fp32.  Q/K/V/out projections are
emitted interleaved into the attention rounds (the PE fills act-engine-bound
gaps); AV for round r runs during round r+1 against the double-buffered
staged s tiles.
"""

import numpy as np

B, N, E, H = 2, 2048, 1024, 16
D = E // H           # 64
NCORES = 8
HG = 4               # head groups
DH = E // HG         # 256 features per head-group
P = 128
NCH = N // 512       # 4 n-chunks of 512
ECH = E // P         # 8 contraction chunks
DCH = DH // P        # 2 feature chunks per shard
KT = N // P          # 16 key tiles
NR = NCH * DCH       # 8 attention rounds (qc, pr)
SCALE_COL = float(E ** 0.5)   # 32.0; column 64 of po = 32*rowsum

# flat fp16 blob layout (element offsets).  x stored n-chunk-major so each
# 512-position chunk is contiguous: [ni][ec][p][m] = xT[ec*128+p, ni*512+m].
SZ_X = E * N          # 2097152 full
SZ_XS = SZ_X // HG    # 524288 per 512-position chunk
SZ_W = E * DH         # 262144
SZ_WP = DH * E        # 262144
OFF_XQ = 0
OFF_XK = OFF_XQ + SZ_X
OFF_XV = OFF_XK + SZ_X
OFF_WQ = OFF_XV + SZ_X
OFF_WK = OFF_WQ + SZ_W
OFF_WV = OFF_WK + SZ_W
OFF_WP = OFF_WV + SZ_W
OFF_BQ = OFF_WP + SZ_WP
OFF_BK = OFF_BQ + DH
OFF_BV = OFF_BK + DH
SZ_BLOB = OFF_BV + DH

_CACHE = {}


def _build_program():
    import concourse.bacc as bacc
    import concourse.tile as tile
    from concourse import mybir

    F32 = mybir.dt.float32
    F16 = mybir.dt.float16
    BF16 = mybir.dt.bfloat16
    EXP = mybir.ActivationFunctionType.Exp

    nc = bacc.Bacc(None, target_bir_lowering=False, debug=False,
                   enable_partition_id=False, num_devices=NCORES)

    blob = nc.declare_dram_parameter("blob", [SZ_BLOB], F16, isOutput=False)
    out = nc.declare_dram_parameter("out", [N, E], F16, isOutput=True)

    # dram views into the blob
    wqt = blob[OFF_WQ : OFF_WQ + SZ_W].rearrange("(c p m) -> p c m", c=ECH, p=P)
    wkt = blob[OFF_WK : OFF_WK + SZ_W].rearrange("(c p m) -> p c m", c=ECH, p=P)
    wvt = blob[OFF_WV : OFF_WV + SZ_W].rearrange("(c p m) -> p c m", c=ECH, p=P)
    wpt = blob[OFF_WP : OFF_WP + SZ_WP].rearrange("(c p m) -> p c m", c=DCH, p=P)
    bqv = blob[OFF_BQ : OFF_BQ + DH].rearrange("(c p) -> p c", p=P)
    bkv = blob[OFF_BK : OFF_BK + DH].rearrange("(c p) -> p c", p=P)
    bvv = blob[OFF_BV : OFF_BV + DH].rearrange("(a m) -> a m", a=1)
    XOFF = {"xq": OFF_XQ, "xk": OFF_XK, "xv": OFF_XV}

    def x_chunk_ap(nm, ni):
        off = XOFF[nm] + ni * SZ_XS
        return blob[off : off + SZ_XS].rearrange("(c p m) -> p c m", c=ECH, p=P)

    with tile.TileContext(nc) as tc:
        with (
            nc.allow_low_precision(reason="16-bit activations; tol 2e-2"),
            tc.tile_pool(name="singles", bufs=1) as singles,
            tc.tile_pool(name="xkp", bufs=4) as xkp,
            tc.tile_pool(name="xqp", bufs=2) as xqp,
            tc.tile_pool(name="xvp", bufs=2) as xvp,
            tc.tile_pool(name="spool", bufs=2) as spool,
            tc.tile_pool(name="npool", bufs=4) as npool,
            tc.tile_pool(name="opool", bufs=4) as opool,
            tc.tile_pool(name="obig", bufs=4) as obig,
            tc.tile_pool(name="pproj", bufs=2, space="PSUM") as pproj,
            tc.tile_pool(name="peps", bufs=2, space="PSUM") as peps,
            tc.tile_pool(name="ppo", bufs=2, space="PSUM") as ppo,
        ):
            # ---- persistent weights / biases / accumulators ----
            wq_sb = singles.tile([P, ECH, DH], F16)
            wk_sb = singles.tile([P, ECH, DH], F16)
            wv_sb = singles.tile([P, ECH, DH], F16)
            wp_sb = singles.tile([P, DCH, E], F16)
            bq16 = singles.tile([P, DCH], F16)
            bk16 = singles.tile([P, DCH], F16)
            bq_sb = singles.tile([P, DCH], F32)
            bk_sb = singles.tile([P, DCH], F32)
            bv_sb = singles.tile([1, DH], F16)
            ones1 = singles.tile([1, P], F16)
            qT_sb = singles.tile([P, DCH, N], F16)
            kT_sb = singles.tile([P, DCH, N], F16)
            oT_sb = singles.tile([P, DCH, N], F16)
            v_sb = singles.tile([P, KT, HG, D + 1], BF16)

            xt = {}     # live x chunk tiles

            def dma_x(nm, ni):
                pool = {"xk": xkp, "xq": xqp, "xv": xvp}[nm]
                t = pool.tile([P, ECH, 512], F16, tag="x", name=f"{nm}{ni}")
                nc.sync.dma_start(out=t, in_=x_chunk_ap(nm, ni))
                xt[(nm, ni)] = t

            # startup DMAs, interleaved halves so the first projection
            # matmuls chase the DMA; later x chunks follow so no unit waits.
            nc.sync.dma_start(out=bk16, in_=bkv)
            nc.sync.dma_start(out=bq16, in_=bqv)
            xk0 = xkp.tile([P, ECH, 512], F16, tag="x", name="xk0")
            xq0 = xqp.tile([P, ECH, 512], F16, tag="x", name="xq0")
            xt[("xk", 0)], xt[("xq", 0)] = xk0, xq0
            xk0s = x_chunk_ap("xk", 0)
            xq0s = x_chunk_ap("xq", 0)
            nc.sync.dma_start(out=wk_sb[:, 0:4, :], in_=wkt[:, 0:4, :])
            nc.sync.dma_start(out=xk0[:, 0:4, :], in_=xk0s[:, 0:4, :])
            nc.sync.dma_start(out=wk_sb[:, 4:8, :], in_=wkt[:, 4:8, :])
            nc.sync.dma_start(out=xk0[:, 4:8, :], in_=xk0s[:, 4:8, :])
            nc.sync.dma_start(out=wq_sb[:, 0:4, :], in_=wqt[:, 0:4, :])
            nc.sync.dma_start(out=xq0[:, 0:4, :], in_=xq0s[:, 0:4, :])
            nc.sync.dma_start(out=wq_sb[:, 4:8, :], in_=wqt[:, 4:8, :])
            nc.sync.dma_start(out=xq0[:, 4:8, :], in_=xq0s[:, 4:8, :])
            nc.vector.tensor_copy(bk_sb, bk16)
            nc.vector.tensor_copy(bq_sb, bq16)
            nc.vector.memset(ones1, 1.0)
            nc.vector.memset(v_sb[:, :, :, D : D + 1], SCALE_COL)
            dma_x("xk", 1)
            nc.sync.dma_start(out=wv_sb, in_=wvt)
            nc.sync.dma_start(out=bv_sb, in_=bvv)
            dma_x("xv", 0)
            dma_x("xk", 2)
            dma_x("xv", 1)
            dma_x("xk", 3)
            nc.sync.dma_start(out=wp_sb, in_=wpt)
            dma_x("xq", 1)

            # one-time bv broadcast to all partitions (saves a matmul per
            # v-projection tile); emitted as a round-0 slot after bv lands
            bvb_sb = singles.tile([P, DH], F32)

            def bv_broadcast():
                bvb_ps = pproj.tile([P, DH], F32, tag="proj", name="bvb_ps")
                nc.tensor.matmul(bvb_ps, ones1, bv_sb, start=True, stop=True)
                nc.vector.tensor_copy(bvb_sb, bvb_ps)

            def kqproj(which, ni, dc):
                w_sb, b_sb, dst = (
                    (wk_sb, bk_sb, kT_sb) if which == "k" else (wq_sb, bq_sb, qT_sb)
                )
                x_c = xt[("x" + which, ni)]
                ps = pproj.tile([P, 512], F32, tag="proj", name=f"{which}ps{ni}{dc}")
                for ec in range(ECH):
                    nc.tensor.matmul(
                        ps,
                        w_sb[:, ec, dc * P : (dc + 1) * P],
                        x_c[:, ec, :],
                        start=(ec == 0),
                        stop=(ec == ECH - 1),
                    )
                nc.vector.tensor_scalar_add(
                    dst[:, dc, ni * 512 : (ni + 1) * 512], ps, b_sb[:, dc : dc + 1]
                )

            def vproj(ni, half):
                x_c = xt[("xv", ni)]
                for k4 in (2 * half, 2 * half + 1):
                    kt = ni * 4 + k4
                    vps = pproj.tile([P, DH], F32, tag="proj", name=f"vps{kt}")
                    for ec in range(ECH):
                        nc.tensor.matmul(
                            vps,
                            x_c[:, ec, k4 * P : (k4 + 1) * P],
                            wv_sb[:, ec, :],
                            start=(ec == 0),
                            stop=(ec == ECH - 1),
                        )
                    nc.vector.tensor_add(
                        v_sb[:, kt, :, 0:D],
                        vps.rearrange("p (h d) -> p h d", h=HG),
                        bvb_sb.rearrange("p (h d) -> p h d", h=HG),
                    )

            def outproj(qc, nt, tail=False):
                n0 = qc * 512 + nt * P
                osb = obig.tile([P, E], F16, tag="osb", name=f"osb{qc}{nt}")
                for ecx in range(2):
                    ops = pproj.tile(
                        [P, 512], F32, tag="proj", name=f"ops{qc}{nt}{ecx}"
                    )
                    for dc in range(DCH):
                        nc.tensor.matmul(
                            ops,
                            oT_sb[:, dc, n0 : n0 + P],
                            wp_sb[:, dc, ecx * 512 : (ecx + 1) * 512],
                            start=(dc == 0),
                            stop=(dc == DCH - 1),
                        )
                    half = osb[:, ecx * 512 : (ecx + 1) * 512]
                    if tail:
                        # tail: copy on the idle ScalarE to keep DVE short
                        nc.scalar.activation(half, ops, COPY)
                    else:
                        nc.vector.tensor_copy(half, ops)
                nc.sync.dma_start(out=out[n0 : n0 + P, :], in_=osb)

            COPY = mybir.ActivationFunctionType.Copy

            # ---- AV for the round (qcp, prp) whose s tiles are staged ----
            def av_unit(qcp, prp, sTprev, pos, qt, h, scalar_evict=False):
                if h == 0:
                    pos[qt] = ppo.tile([P, 2 * (D + 1)], F32, tag="po",
                                       name=f"po{qcp}{prp}{qt}")
                po = pos[qt]
                for kt in range(h * 8, h * 8 + 8):
                    ktg, j = kt // 2, kt % 2
                    for hp in range(2):
                        # one accumulation group per po bank: start only on
                        # the very first write (zeroes the whole 2KB zero
                        # region lazily), stop on the very last.
                        nc.tensor.matmul(
                            po[:, hp * (D + 1) : (hp + 1) * (D + 1)],
                            sTprev[(ktg, hp)][
                                :, j * 512 + qt * P : j * 512 + (qt + 1) * P
                            ],
                            v_sb[:, kt, 2 * prp + hp, :],
                            start=(kt == 0 and hp == 0),
                            stop=(kt == KT - 1 and hp == 1),
                            skip_group_check=True,
                        )
                if h == 1:
                    o_t = opool.tile([P, P], F16, tag="o", name=f"o{qcp}{prp}{qt}")
                    for hp in range(2):
                        rinv = npool.tile([P, 1], F32, tag="rinv",
                                          name=f"ri{qcp}{prp}{qt}{hp}")
                        c0 = hp * (D + 1)
                        nc.vector.reciprocal(rinv, po[:, c0 + D : c0 + D + 1])
                        if scalar_evict:
                            # tail: ScalarE is idle; keep DVE off the chain
                            nc.scalar.activation(
                                o_t[:, hp * D : (hp + 1) * D], po[:, c0 : c0 + D],
                                COPY, scale=rinv,
                            )
                        else:
                            nc.vector.tensor_scalar_mul(
                                o_t[:, hp * D : (hp + 1) * D], po[:, c0 : c0 + D], rinv
                            )
                    # tail transposes ride the Activation hwdge queue so they
                    # never wait behind out-DMAs on the SP queue
                    dma_eng = nc.scalar if scalar_evict else nc.sync
                    dma_eng.dma_start_transpose(
                        out=oT_sb[:, prp, qcp * 512 + qt * P : qcp * 512 + (qt + 1) * P],
                        in_=o_t,
                    )

            def av_units(qcp, prp, sTprev, scalar_evict=False):
                pos = {}
                return [
                    (lambda qt=qt, h=h: av_unit(qcp, prp, sTprev, pos, qt, h,
                                                scalar_evict))
                    for qt in range(4) for h in range(2)
                ]

            # ---- one attention round: energies+exp, slots interleaved.
            # The energy pair for unit g+1 is emitted BEFORE slot g's filler
            # work so the in-order PE queue always has the next eps ready
            # for ScalarE even when a slot unit runs long. ----
            def emit_round(r, qc, pr, slots):
                cur = {}

                def energy(g):
                    ktg, hp = g // 2, g % 2
                    rows = slice(hp * D, (hp + 1) * D)
                    eps = peps.tile([P, 1024], F32, tag="eps", name=f"eps{r}{g}")
                    for j in range(2):
                        kt = ktg * 2 + j
                        nc.tensor.matmul(
                            eps[:, j * 512 : (j + 1) * 512],
                            kT_sb[rows, pr, kt * P : (kt + 1) * P],
                            qT_sb[rows, pr, qc * 512 : (qc + 1) * 512],
                            start=True,
                            stop=True,
                        )
                    sT = spool.tile([P, 1024], BF16, tag=f"sT{g}", name=f"sT{r}{g}")
                    nc.scalar.activation(sT, eps, EXP)
                    cur[(ktg, hp)] = sT

                energy(0)
                for g in range(16):
                    if g + 1 < 16:
                        energy(g + 1)
                    for u in slots.get(g, []):
                        u()
                return cur

            # ---- prefix: projections needed by round 0 ----
            kqproj("k", 0, 0)
            kqproj("q", 0, 0)

            # ---- round 0 (qc0, pr0): fill with k dc0 chain + v chunks 0,1 ----
            s_prev = emit_round(0, 0, 0, {
                2: [lambda: kqproj("k", 1, 0), bv_broadcast],
                3: [lambda: vproj(0, 0)],
                4: [lambda: vproj(0, 1)],
                6: [lambda: kqproj("k", 2, 0)],
                7: [lambda: kqproj("q", 0, 1)],
                8: [lambda: vproj(1, 0)],
                10: [lambda: kqproj("k", 3, 0)],
                11: [lambda: vproj(1, 1)],
                12: [lambda: dma_x("xv", 2)],
                13: [lambda: kqproj("k", 0, 1)],
                15: [lambda: dma_x("xv", 3)],
            })

            # ---- round 1 (qc0, pr1): v chunks 2,3 + k dc1 + AV(0,0) ----
            av = av_units(0, 0, s_prev)
            s_prev = emit_round(1, 0, 1, {
                0: [lambda: vproj(2, 0)],
                1: [lambda: vproj(2, 1)],
                2: [lambda: kqproj("k", 1, 1)],
                3: [lambda: vproj(3, 0)],
                4: [lambda: vproj(3, 1)],
                5: [av[0]],            # qt0 kt0-7
                6: [lambda: kqproj("k", 2, 1)],
                7: [av[2]],            # qt1 kt0-7
                9: [av[1]],            # qt0 kt8-15 + evict
                10: [lambda: kqproj("k", 3, 1)],
                11: [av[3]],           # qt1 kt8-15 + evict
                12: [lambda: kqproj("q", 1, 0)],
                13: [av[4], av[5]],    # qt2
                14: [av[6]],
                15: [av[7]],           # qt3 + evict
            })

            # ---- steady rounds r=2..7 ----
            for r in range(2, NR):
                qc, pr = r // 2, r % 2
                qcp, prp = (r - 1) // 2, (r - 1) % 2
                av = av_units(qcp, prp, s_prev)
                slots = {1: [av[0]], 3: [av[1]], 5: [av[2]], 7: [av[3]],
                         9: [av[4]], 11: [av[5]], 13: [av[6]], 14: [av[7]]}
                if pr == 0:
                    # q(qc, dc1) due next round; xq(qc+1) due round r+1 g12
                    slots.setdefault(0, []).append(
                        lambda qc=qc: kqproj("q", qc, 1))
                    if qc + 1 < NCH:
                        slots.setdefault(2, []).append(
                            lambda qc=qc: dma_x("xq", qc + 1))
                else:
                    if qc + 1 < NCH:
                        slots.setdefault(12, []).append(
                            lambda qc=qc: kqproj("q", qc + 1, 0))
                    # outproj(qc-1): oT(qc-1) evicts landed during round r-1
                    oqc = qc - 1
                    slots.setdefault(0, []).append(lambda oqc=oqc: outproj(oqc, 0))
                    slots.setdefault(4, []).append(lambda oqc=oqc: outproj(oqc, 1))
                    slots.setdefault(8, []).append(lambda oqc=oqc: outproj(oqc, 2))
                    slots.setdefault(10, []).append(lambda oqc=oqc: outproj(oqc, 3))
                s_prev = emit_round(r, qc, pr, slots)

            # ---- tail: AV of the last round + final out-projection,
            # interleaved so each qt's outproj chases its evict+transpose ----
            av = av_units(NCH - 1, 1, s_prev, scalar_evict=True)
            av[0](); av[1]()
            av[2](); av[3]()
            outproj(NCH - 1, 0, tail=True)
            av[4](); av[5]()
            outproj(NCH - 1, 1, tail=True)
            av[6](); av[7]()
            outproj(NCH - 1, 2, tail=True)
            outproj(NCH - 1, 3, tail=True)

    nc.compile()
    return nc


def _shard_inputs(queries, keys, values, Wq, bq, Wk, bk, Wv, bv, Wp):
    """Host-side shard/layout prep: one flat fp16 blob per core holding
    the full feature-major activations for its batch (n-chunk-major),
    transposed weight shards, and biases."""
    f32 = np.float32
    f16 = np.float16
    # full x per batch: [ni][ec][p][m] = x[b].T[ec*128+p, ni*512+m]
    xCM = {}
    for name, x in (("xq", queries), ("xk", keys), ("xv", values)):
        xCM[name] = [
            np.ascontiguousarray(
                np.asarray(x[b], f32).T.astype(f16)
                .reshape(ECH, P, NCH, 512).transpose(2, 0, 1, 3)
            ).ravel()
            for b in range(B)
        ]
    Wq, Wk, Wv = (np.asarray(w, f32) for w in (Wq, Wk, Wv))
    Wp = np.asarray(Wp, f32)
    bq, bk, bv = (np.asarray(b_, f32) for b_ in (bq, bk, bv))
    maps = []
    for c in range(NCORES):
        b, hg = c // HG, c % HG
        rows = slice(hg * DH, (hg + 1) * DH)
        blob = np.concatenate([
            xCM["xq"][b], xCM["xk"][b], xCM["xv"][b],
            np.ascontiguousarray(Wq[rows].T.astype(f16)).ravel(),
            np.ascontiguousarray(Wk[rows].T.astype(f16)).ravel(),
            np.ascontiguousarray(Wv[rows].T.astype(f16)).ravel(),
            np.ascontiguousarray(Wp[:, rows].T.astype(f16)).ravel(),
            bq[rows].astype(f16), bk[rows].astype(f16), bv[rows].astype(f16),
        ])
        assert blob.shape[0] == SZ_BLOB
        maps.append({"blob": blob})
    return maps


def kernel(queries, keys, values, Wq, bq, Wk, bk, Wv, bv, Wp, bp):
    from concourse.bass_utils import run_bass_kernel_spmd

    if "nc" not in _CACHE:
        _CACHE["nc"] = _build_program()
    nc = _CACHE["nc"]

    in_maps = _shard_inputs(queries, keys, values, Wq, bq, Wk, bk, Wv, bv, Wp)

    res = run_bass_kernel_spmd(nc, in_maps, list(range(NCORES)))

    out = np.zeros((B, N, E), np.float32)
    for c in range(NCORES):
        out[c // HG] += res.results[c]["out"].astype(np.float32)
    out += np.asarray(bp, np.float32)
    return out
